# revision 12
# baseline (speedup 1.0000x reference)
"""CapsuleLayer (dynamic routing, ROUTING_ITER=2) Bass/Tile kernel for TRN2.

Contract: kernel(x, weight) takes FULL inputs
  x:      [64, 2048, 1, 16] f32
  weight: [1, 2048, 32, 16, 16] f32
returns FULL output [64, 32, 16] f32.

Sharding: data-parallel over batch B=64 across 8 cores (8 per core),
weight replicated. Self-contained: hardcodes shapes, imports only
numpy/ml_dtypes/concourse.

Engine split: PE does u = W@x and all s-accumulations; Act does the
PSUM->SBUF u copies and exp; routing elementwise chunks are distributed
DVE (bf16 2x mode) vs Pool (scalar_tensor_tensor path).
"""

from contextlib import ExitStack

import ml_dtypes
import numpy as np

import concourse.bacc as bacc
import concourse.bass as bass
import concourse.mybir as mybir
import concourse.tile as tile
from concourse.bass_utils import run_bass_kernel_spmd

F32 = mybir.dt.float32
BF16 = mybir.dt.bfloat16
AF = mybir.ActivationFunctionType
AX = mybir.AxisListType
ALU = mybir.AluOpType

EPS = 1e-8
J, D, E = 32, 16, 16
JD = J * D  # 512


def emit_capsule(tc, w2, xbd, d1, ds, out, n_in, b_loc=8):
    """Emit the per-core capsule program.

    DRAM tensors (APs):
      w2  [G, 8, E, JD] bf16  weight, host-permuted to [i, e, d, j], i=g*8+di
      xbd [128, G*64]   bf16  block-diag x stationary, partition-major
      d1  [128, 8] bf16       delta matrix * (1/32)  (s1 accumulation)
      ds  [128, 8] bf16       delta matrix * 1.0     (s2/s3 accumulation)
      out [b_loc, JD] f32     squash(s3) output, (j,d) layout

    u layout in SBUF (bf16): partition p = (g%2)*64 + b*8 + di,
    free f = (g//2)*JD + d*J + j, where i = g*8 + di.
    """
    nc = tc.nc
    assert b_loc == 8
    G = n_in // 8
    GH = G // 2
    GDMA = 8   # groups per W DMA chunk
    CH = 8     # gh per routing chunk
    assert G % GDMA == 0 and GH % CH == 0
    NCH = GH // CH
    POOL_OWN = set(range(4, NCH, 5))  # routing chunks owned by Pool engine

    ctx = ExitStack()
    singles = ctx.enter_context(tc.tile_pool(name="singles", bufs=1))
    small = ctx.enter_context(tc.tile_pool(name="small", bufs=2))
    dramp = ctx.enter_context(tc.tile_pool(name="dram_scratch", bufs=1, space="DRAM"))

    u_sb = singles.tile([128, GH * JD], BF16)
    d1_sb = singles.tile([128, 8], BF16)
    ds_sb = singles.tile([128, 8], BF16)
    nc.sync.dma_start(out=d1_sb, in_=d1)
    nc.sync.dma_start(out=ds_sb, in_=ds)
    v_exp = singles.tile([128, JD], BF16)
    V = singles.tile([8, JD], F32)      # running sum of v vectors
    s_sb = singles.tile([8, JD], F32)
    vscr = dramp.tile([8, JD], BF16)    # DRAM bounce buffer for v broadcast
    eps8 = singles.tile([8, 1], F32)
    nc.vector.memset(eps8, EPS)

    # ---------- squash helpers (all on 8 partitions, tiny) ----------
    def squash_j(s_in, v_out):
        # v = squash(s, axis=j):  sq[b,d] = sum_j s^2;  v = s*sq/((1+sq)*sqrt(sq+eps))
        t2 = small.tile([8, JD], F32, tag="sqt2")
        nc.vector.tensor_mul(t2, s_in, s_in)
        sv = small.tile([8, 4, J], F32, tag="sqv")
        sq, a, t3, w = sv[:, 0, :D], sv[:, 1, :D], sv[:, 2, :D], sv[:, 3, :D]
        nc.vector.reduce_sum(out=sq, in_=t2.rearrange("p (d j) -> p d j", d=D), axis=AX.X)
        nc.scalar.activation(a, sq, AF.Sqrt, bias=eps8)
        nc.vector.tensor_mul(t3, sq, a)
        nc.vector.tensor_add(t3, t3, a)          # a*(1+sq)
        nc.vector.reciprocal(w, t3)
        nc.vector.tensor_mul(w, w, sq)           # sq/((1+sq)a)
        wb = w.unsqueeze(2).broadcast_to([8, D, J])
        nc.vector.tensor_mul(v_out.rearrange("p (d j) -> p d j", d=D),
                             s_in.rearrange("p (d j) -> p d j", d=D), wb)

    def squash_d(s_in, v_out):
        # v = squash(s, axis=d): sq[b,j] = sum_d s^2
        t2 = small.tile([8, JD], F32, tag="sqt2")
        nc.vector.tensor_mul(t2, s_in, s_in)
        sv = small.tile([8, 4, J], F32, tag="sqv")
        sq, a, t3, w = sv[:, 0, :], sv[:, 1, :], sv[:, 2, :], sv[:, 3, :]
        nc.vector.reduce_sum(out=sq, in_=t2.rearrange("p (d j) -> p j d", d=D), axis=AX.X)
        nc.scalar.activation(a, sq, AF.Sqrt, bias=eps8)
        nc.vector.tensor_mul(t3, sq, a)
        nc.vector.tensor_add(t3, t3, a)
        nc.vector.reciprocal(w, t3)
        nc.vector.tensor_mul(w, w, sq)
        wb = w.unsqueeze(1).broadcast_to([8, D, J])
        nc.vector.tensor_mul(v_out.rearrange("p (d j) -> p d j", d=D),
                             s_in.rearrange("p (d j) -> p d j", d=D), wb)

    def refresh_v_exp():
        vb = small.tile([8, JD], BF16, tag="vb")
        nc.vector.tensor_copy(out=vb, in_=V)
        nc.sync.dma_start(out=vscr, in_=vb)
        src = vscr.unsqueeze(1).broadcast_to([8, 8, JD])
        for g2 in range(2):
            nc.sync.dma_start(out=v_exp[g2 * 64:(g2 + 1) * 64, :], in_=src)

    # ---------- phase 1: W pass (u = W @ x), s1 accumulation ----------
    with tc.tile_pool(name="xsb", bufs=1) as xsb_pool, \
         tc.tile_pool(name="wp", bufs=3) as wp, \
         tc.tile_pool(name="up", bufs=3, space="PSUM") as up, \
         tc.tile_pool(name="sp", bufs=1, space="PSUM") as sp:
        xbd_sb = xsb_pool.tile([128, G * 64], BF16)
        nc.sync.dma_start(out=xbd_sb, in_=xbd)
        s1_ps = sp.tile([8, JD], F32)
        for ci in range(G // GDMA):
            wt = wp.tile([128, GDMA, JD], BF16, tag="wt")
            # src: dims (k=(di,e) merged, g, jd)
            wsrc = w2[ci * GDMA:(ci + 1) * GDMA].rearrange("g di e f -> (di e) g f")
            nc.sync.dma_start(out=wt, in_=wsrc)
            for gq in range(GDMA // 4):
                pt = up.tile([128, 2 * JD], F32, tag="upt")
                for idx in range(4):
                    gl = gq * 4 + idx
                    g_abs = ci * GDMA + gl
                    nc.tensor.matmul(
                        pt[(gl % 2) * 64:(gl % 2) * 64 + 64,
                           (idx // 2) * JD:(idx // 2) * JD + JD],
                        xbd_sb[:, g_abs * 64:(g_abs + 1) * 64],
                        wt[:, gl, :], start=True, stop=True)
                gh0 = ci * (GDMA // 2) + gq * 2
                nc.scalar.copy(out=u_sb[:, gh0 * JD:(gh0 + 2) * JD], in_=pt)
                for gh in (gh0, gh0 + 1):
                    nc.tensor.matmul(s1_ps, d1_sb, u_sb[:, gh * JD:(gh + 1) * JD],
                                     start=(gh == 0), stop=(gh == GH - 1))
        nc.vector.tensor_copy(out=s_sb, in_=s1_ps)

    squash_j(s_sb, V)      # V = v1
    refresh_v_exp()

    # ---------- routing pass (T = u.V, softmax, s = sum_i c*u) ----------
    def routing_pass(final):
        with tc.tile_pool(name="rv", bufs=2) as rv, \
             tc.tile_pool(name="rg", bufs=1) as rg, \
             tc.tile_pool(name="spp", bufs=1, space="PSUM") as spp:
            s_ps = spp.tile([8, JD], F32)
            for k in range(NCH):
                pool_own = k in POOL_OWN
                pp = rg if pool_own else rv
                gh0 = k * CH
                fs, fe = gh0 * JD, (gh0 + CH) * JD
                u_ch = u_sb[:, fs:fe].rearrange("p (g f) -> p g f", g=CH)
                u4 = u_ch.rearrange("p g (d j) -> p g d j", d=D)
                prod = pp.tile([128, CH, JD], BF16, tag="prod")
                vb = v_exp.unsqueeze(1).broadcast_to([128, CH, JD])
                eng = nc.gpsimd if pool_own else nc.vector
                eng.tensor_mul(prod, u_ch, vb)
                # tree-reduce over d (outer dim: 16 -> 8 -> 4 -> 2 -> 1), bf16
                p4 = prod.rearrange("p g (d j) -> p g d j", d=D)
                t1 = pp.tile([128, CH, 8, J], BF16, tag="t1")
                t2 = pp.tile([128, CH, 4, J], BF16, tag="t2")
                t3 = pp.tile([128, CH, 2, J], BF16, tag="t3")
                tt = pp.tile([128, CH, J], BF16, tag="tt")
                eng.tensor_add(t1, p4[:, :, 0:8, :], p4[:, :, 8:16, :])
                eng.tensor_add(t2, t1[:, :, 0:4, :], t1[:, :, 4:8, :])
                eng.tensor_add(t3, t2[:, :, 0:2, :], t2[:, :, 2:4, :])
                eng.tensor_add(tt.unsqueeze(2),
                               t3[:, :, 0:1, :], t3[:, :, 1:2, :])
                # softmax over j (no max subtraction; logits are tiny)
                eT = pp.tile([128, CH, J], BF16, tag="eT")
                nc.scalar.activation(eT, tt, AF.Exp)
                se = pp.tile([128, CH], F32, tag="se")
                r = pp.tile([128, CH], F32, tag="r")
                c = pp.tile([128, CH, J], BF16, tag="c")
                rb = r.unsqueeze(2).broadcast_to([128, CH, J])
                cb = c.unsqueeze(2).broadcast_to([128, CH, D, J])
                nc.vector.reduce_sum(out=se, in_=eT, axis=AX.X)
                nc.vector.reciprocal(r, se)
                eng.tensor_mul(c, eT, rb)
                # y = c (broadcast over d) * u, written over prod (dead after t1)
                eng.tensor_mul(p4, u4, cb)
                for q in range(CH):
                    gh = gh0 + q
                    nc.tensor.matmul(s_ps, ds_sb, prod[:, q, :],
                                     start=(gh == 0), stop=(gh == GH - 1))
            nc.vector.tensor_copy(out=s_sb, in_=s_ps)
        if not final:
            v2 = small.tile([8, JD], F32, tag="v2")
            squash_j(s_sb, v2)
            nc.vector.tensor_add(V, V, v2)
            refresh_v_exp()
        else:
            vout = small.tile([8, JD], F32, tag="vout")
            squash_d(s_sb, vout)
            nc.sync.dma_start(out=out, in_=vout)

    routing_pass(final=False)   # iteration 2 (uses V=v1)
    routing_pass(final=True)    # final (uses V=v1+v2)
    ctx.close()


def build_module(n_in=2048, b_loc=8, num_devices=8, enable_asserts=False):
    nc = bacc.Bacc("TRN2", target_bir_lowering=False, debug=False,
                   num_devices=num_devices, enable_asserts=enable_asserts)
    G = n_in // 8
    w2 = nc.dram_tensor("w2", [G, 8, E, JD], BF16, kind="ExternalInput").ap()
    xbd = nc.dram_tensor("xbd", [128, G * 64], BF16, kind="ExternalInput").ap()
    d1 = nc.dram_tensor("d1", [128, 8], BF16, kind="ExternalInput").ap()
    ds = nc.dram_tensor("ds", [128, 8], BF16, kind="ExternalInput").ap()
    out = nc.dram_tensor("out", [b_loc, JD], F32, kind="ExternalOutput").ap()
    with tile.TileContext(nc) as tc:
        emit_capsule(tc, w2, xbd, d1, ds, out, n_in=n_in, b_loc=b_loc)
    nc.compile()
    return nc


def host_prep_w(weight, n_in):
    # weight [1, N, J, D, E] -> w2 [G, 8, E, J*D] with free layout (d, j)
    w2 = np.ascontiguousarray(weight[0].transpose(0, 3, 2, 1))  # [N, E, D, J]
    return w2.reshape(n_in // 8, 8, E, JD).astype(ml_dtypes.bfloat16)


def host_prep_xbd(xs, n_in):
    # xs [b_loc, N, E] -> xbd [128, G*64] block-diagonal stationary,
    # partition-major: partition k=(di,e), free (g, b, di')
    G = n_in // 8
    t = xs.reshape(8, G, 8, E).transpose(2, 3, 1, 0)  # [di, e, G, b]
    xbd = np.zeros((8, E, G, 8, 8), np.float32)       # [di, e, G, b, di']
    for di in range(8):
        xbd[di, :, :, :, di] = t[di]
    return xbd.reshape(128, G * 64).astype(ml_dtypes.bfloat16)


def host_prep_deltas():
    p = np.arange(128)
    bofp = (p // 8) % 8
    d1 = np.zeros((128, 8), np.float32)
    ds = np.zeros((128, 8), np.float32)
    d1[p, bofp] = 1.0 / 32.0
    ds[p, bofp] = 1.0
    return d1.astype(ml_dtypes.bfloat16), ds.astype(ml_dtypes.bfloat16)


_CACHE = {}
LAST_EXEC_NS = None


def kernel(x, weight, trace=False):
    B, N_in = 64, 2048
    n_cores = 8
    b_loc = B // n_cores
    key = (N_in, b_loc, n_cores)
    if key not in _CACHE:
        _CACHE[key] = build_module(n_in=N_in, b_loc=b_loc, num_devices=n_cores)
    nc = _CACHE[key]

    x = np.asarray(x, dtype=np.float32)
    weight = np.asarray(weight, dtype=np.float32)
    w2 = host_prep_w(weight, N_in)
    d1, ds = host_prep_deltas()
    in_maps = []
    for c in range(n_cores):
        xs = np.ascontiguousarray(x[c * b_loc:(c + 1) * b_loc, :, 0, :])
        in_maps.append({
            "w2": w2,
            "xbd": host_prep_xbd(xs, N_in),
            "d1": d1,
            "ds": ds,
        })
    global LAST_EXEC_NS
    res = run_bass_kernel_spmd(nc, in_maps, core_ids=list(range(n_cores)),
                               trace=trace)
    LAST_EXEC_NS = res.exec_time_ns
    outs = [r["out"].reshape(b_loc, D, J).transpose(0, 2, 1) for r in res.results]
    return np.ascontiguousarray(np.concatenate(outs, axis=0))
